# revision 17
# baseline (speedup 1.0000x reference)
import sys

sys.path.insert(0, "/opt/trn_rl_repo")

import ml_dtypes
import numpy as np

import concourse.bacc as bacc
import concourse.bass as bass
import concourse.mybir as mybir
import concourse.tile as tile
from concourse.bass_utils import run_bass_kernel_spmd

F32 = mybir.dt.float32
BF16 = mybir.dt.bfloat16
BF_NP = ml_dtypes.bfloat16

N, M, G, A, H = 20000, 48, 16, 64, 16
NCORES = 8
NL = N // NCORES  # 2500 atoms per core
NPAIRS = NL // 2  # 1250; atom n = 2p + s  (s = col-tile set)
GP = 256  # pairs per group (512 atoms): stage-2/epilogue granularity
BLK = 64  # pairs per input DMA block (128 atoms)
FL = 8  # pairs per psum1 flush (16 atoms, 1 psum bank)

# vbig partition row map: set s at rows 64s..64s+64:
#   rows 64s+0:16   out_s^T (g)
#   rows 64s+16+16d : 16  t d-slice (g)
# Stage-2 matmuls use K=32 (a full 32-row group) with zero-padded weights so
# lhsT/rhs partition bases stay 32-aligned (walrus requires base == tile row
# base). Row-group base per (s, d); weight variant (column offset) per d:
# A (offset 0) holds agh at the d0/d1 row offsets, B (offset 1536) at d2's.
RB32 = {(0, 0): 0, (0, 1): 32, (0, 2): 32, (1, 0): 64, (1, 1): 96, (1, 2): 96}
WOFF = {0: 0, 1: 0, 2: 1536}  # d -> column offset into aghw
AGHW_COLS = 3072  # two variants x 32 pairs x 48 cols ([agh_a |0| agh_b])

_nc_cache = {}


def _dummy_mm(nc, out_ap, dep_ap):
    # 1x1 matmul whose only job is to absorb one semaphore wait (walrus
    # encodes <=1 sync-wait per PE instruction); the dependency is expressed
    # through its operands (dep_ap must sit at partition base 0).
    nc.tensor.matmul(
        out=out_ap, lhsT=dep_ap, rhs=dep_ap, start=True, stop=True,
        tile_position=(0, 0),
    )


def _build(nl=NL, sim=False):
    """Per-core Bass program (bf16 end to end, fp32 PSUM accumulate).

    Stage 1 (per atom): one matmul, lhsT = acat cols 64:128 ([gs16|gv48]),
      rhs = acat cols 0:64 (a). out[0:16]=out_s^T, out[16:64]=t, with even
      atoms on out partitions 0:64 and odd on 64:128 (PE col tiling), so the
      PSUM->SBUF copy runs on all 128 partitions.
    Stage 2 (per 256-pair group, per channel/set/d): K=16 matmul of agh
      against vbig t-rows, 12-way tile_position packing, f = pairs.
    Epilogue: ACT Square psum->sbuf bf16, 2 DVE adds (d-sum), DMA out.
    """
    nc = bacc.Bacc("TRN2", target_bir_lowering=False)
    npairs = nl // 2
    ac_d = nc.declare_dram_parameter("ac", [M, nl, 128], BF16, isOutput=False)
    w_d = nc.declare_dram_parameter("aghw", [128, AGHW_COLS], BF16, isOutput=False)
    outs_d = nc.declare_dram_parameter(
        "outs", [2, G, npairs, A], BF16, isOutput=True
    )
    outv_d = nc.declare_dram_parameter(
        "outv", [2, A, H, npairs], BF16, isOutput=True
    )
    Sq = mybir.ActivationFunctionType.Square

    groups = []
    p = 0
    while p < npairs:
        groups.append((p, min(GP, npairs - p)))
        p += GP

    with tile.TileContext(nc) as tc:
        with (
            tc.tile_pool(name="singles", bufs=1) as singles,
            tc.tile_pool(name="acs", bufs=2) as acs_pool,
            tc.tile_pool(name="vbig", bufs=2) as vbig_pool,
            tc.tile_pool(name="sq", bufs=2) as sq_pool,
            tc.tile_pool(name="ov", bufs=2) as ov_pool,
            tc.tile_pool(name="psum1", bufs=2, space="PSUM") as p1_pool,
            tc.tile_pool(name="psum2", bufs=2, space="PSUM") as p2_pool,
        ):
            aghw = singles.tile([128, AGHW_COLS], BF16)
            nc.sync.dma_start(out=aghw[:, :], in_=w_d[:, :])

            def stage1(p0, npg):
                vbig = vbig_pool.tile([128, GP * A], BF16)
                for b0 in range(0, npg, BLK):
                    nb = min(BLK, npg - b0)
                    acs = acs_pool.tile([128, BLK * 2 * 128], BF16)
                    nc.sync.dma_start(
                        out=acs[0:M, 0 : nb * 256],
                        in_=ac_d[:, (p0 + b0) * 2 : (p0 + b0 + nb) * 2, :].rearrange(
                            "m n c -> m (n c)"
                        ),
                    )
                    for f0 in range(0, nb, FL):
                        nf = min(FL, nb - f0)
                        psum1 = p1_pool.tile([128, FL * A], F32)
                        if f0 == 0:
                            # absorb the psum1 WAR wait into a dummy (its
                            # aghw dep is wait-dominated after the first
                            # use), so the first real matmul's single wait
                            # is the fresh acs-DMA semaphore
                            _dummy_mm(nc, psum1[0:1, 0:1], aghw[0:1, 0:1])
                        for j in range(nf):
                            col = (f0 + j) * 256
                            nc.tensor.matmul(
                                out=psum1[0:64, j * A : (j + 1) * A],
                                lhsT=acs[0:M, col + 64 : col + 128],
                                rhs=acs[0:M, col : col + 64],
                                start=True,
                                stop=True,
                            )
                            nc.tensor.matmul(
                                out=psum1[64:128, j * A : (j + 1) * A],
                                lhsT=acs[0:M, col + 192 : col + 256],
                                rhs=acs[0:M, col + 128 : col + 192],
                                start=True,
                                stop=True,
                            )
                        nc.vector.tensor_copy(
                            out=vbig[:, (b0 + f0) * A : (b0 + f0 + nf) * A],
                            in_=psum1[:, 0 : nf * A],
                        )
                # out_s: vbig rows {0:16, 64:80} -> [2, G, npairs, A]
                for s2 in range(2):
                    nc.sync.dma_start(
                        out=outs_d[s2, :, p0 : p0 + npg, :],
                        in_=vbig[64 * s2 : 64 * s2 + 16, 0 : npg * A].rearrange(
                            "g (p a) -> g p a", a=A
                        ),
                    )
                return vbig

            def stage2(p0, npg, vbig, first_group):
                ov = ov_pool.tile([128, 16 * GP], BF16)
                for q in range(8):
                    for s in range(2):
                        # d-slices at 512-col offsets: one PSUM bank per d,
                        # so concurrent row-group tiles drain to distinct
                        # banks (same-bank cross-row-group drains are fatal)
                        psum2 = p2_pool.tile([128, 3 * 512], F32)
                        if q == 0 and s == 0:
                            # absorb the psum2 WAR wait (square of 2 iters
                            # ago) into a dummy targeting this psum2 tile, so
                            # the first real matmul's single wait is the
                            # vbig-copies-done semaphore. The aghw read also
                            # pins the (once-satisfied) aghw-DMA dep.
                            _dummy_mm(nc, psum2[0:1, 0:1], aghw[0:1, 0:1])
                        for d in range(3):
                            rb = RB32[(s, d)]
                            woff = WOFF[d]
                            for k in range(4):  # channel pairs in this q
                                c = 32 * k
                                pair = 4 * q + k
                                a0 = 8 * q + 2 * k  # alpha channel
                                out_ap = psum2[c : c + 32, d * 512 : d * 512 + npg]
                                rhs3 = vbig[rb : rb + 32, 0 : npg * A].rearrange(
                                    "r (p a) -> r p a", a=A
                                )
                                nc.tensor.matmul(
                                    out=out_ap,
                                    lhsT=aghw[
                                        rb : rb + 32,
                                        woff + 48 * pair : woff + 48 * pair + 32,
                                    ],
                                    rhs=rhs3[:, :, a0],
                                    start=True,
                                    stop=False,
                                    tile_position=(rb, c),
                                )
                                nc.tensor.matmul(
                                    out=out_ap,
                                    lhsT=aghw[
                                        rb : rb + 32,
                                        woff + 48 * pair + 16 : woff + 48 * pair + 48,
                                    ],
                                    rhs=rhs3[:, :, a0 + 1],
                                    start=False,
                                    stop=True,
                                    tile_position=(rb, c),
                                )
                        sq = sq_pool.tile([128, 3 * GP], BF16)
                        nc.scalar.activation(
                            out=sq[:, 0 : 3 * npg],
                            in_=psum2[:, :].rearrange(
                                "p (d z) -> p d z", z=512
                            )[:, :, 0:npg],
                            func=Sq,
                        )
                        ovs = ov[:, (q * 2 + s) * npg : (q * 2 + s + 1) * npg]
                        nc.vector.tensor_add(
                            ovs, sq[:, 0:npg], sq[:, npg : 2 * npg]
                        )
                        nc.vector.tensor_add(
                            ovs, ovs, sq[:, 2 * npg : 3 * npg]
                        )
                for s in range(2):
                    dst = outv_d[s, :, :, p0 : p0 + npg].rearrange(
                        "(q j) h n -> (j h) q n", j=8
                    )
                    src = ov[:, 0 : 16 * npg].rearrange(
                        "p (q s n) -> p q s n", q=8, s=2
                    )[:, :, s, :]
                    nc.sync.dma_start(out=dst, in_=src)

            with nc.allow_low_precision("bf16 stores; fp32 psum accumulate"):
                prev = None
                for gi, (p0, npg) in enumerate(groups):
                    vb = stage1(p0, npg)
                    if prev is not None:
                        stage2(*prev, first_group=(gi == 1))
                    prev = (p0, npg, vb)
                stage2(*prev, first_group=(len(groups) == 1))
    nc.compile()
    return nc


def _get_nc():
    if "nc" not in _nc_cache:
        _nc_cache["nc"] = _build()
    return _nc_cache["nc"]


def _prep(a, gs, gv, agh):
    a = np.asarray(a, np.float32)
    gs = np.asarray(gs, np.float32)
    gv = np.asarray(gv, np.float32)
    agh = np.asarray(agh, np.float32)
    n = a.shape[0]
    ac = np.empty((M, n, 128), BF_NP)
    ac[:, :, 0:64] = a.transpose(1, 0, 2)
    ac[:, :, 64:80] = gs.transpose(1, 0, 2)
    # c = 80 + 16d + g  (d-major)
    ac[:, :, 80:128] = gv.transpose(1, 0, 3, 2).reshape(M, n, 48)
    # aghw [128, 3072]: variant A cols [0:1536] (d0/d1 row offsets), variant
    # B cols [1536:3072] (d2). Per channel pair: [agh_a(16) | 0(16) | agh_b].
    blkcols = np.zeros((G, 1536), np.float32)
    for pair in range(32):
        blkcols[:, 48 * pair : 48 * pair + 16] = agh[2 * pair]
        blkcols[:, 48 * pair + 32 : 48 * pair + 48] = agh[2 * pair + 1]
    w = np.zeros((128, AGHW_COLS), BF_NP)
    for r in (16, 32, 80, 96):  # variant A: d0 (offset 16), d1 (offset 0)
        w[r : r + 16, 0:1536] = blkcols
    for r in (48, 112):  # variant B: d2 (offset 16 in grp1/grp3)
        w[r : r + 16, 1536:3072] = blkcols
    return ac, w


def _assemble(res_core, nl):
    npairs = nl // 2
    o_s = (
        np.asarray(res_core["outs"])
        .astype(np.float32)
        .transpose(2, 0, 3, 1)
        .reshape(nl, A * G)
    )
    o_v = (
        np.asarray(res_core["outv"])
        .astype(np.float32)
        .transpose(3, 0, 1, 2)
        .reshape(nl, A * H)
    )
    return np.concatenate([o_s, o_v], axis=1)


def kernel(a, gs, gv, agh):
    ac, w = _prep(a, gs, gv, agh)
    nc = _get_nc()
    in_maps = [
        {"ac": np.ascontiguousarray(ac[:, c * NL : (c + 1) * NL, :]), "aghw": w}
        for c in range(NCORES)
    ]
    res = run_bass_kernel_spmd(nc, in_maps, list(range(NCORES))).results
    return np.concatenate(
        [_assemble(res[c], NL) for c in range(NCORES)], axis=0
    )


# revision 25
# speedup vs baseline: 1.0015x; 1.0015x over previous
import sys

sys.path.insert(0, "/opt/trn_rl_repo")

import ml_dtypes
import numpy as np

import concourse.bacc as bacc
import concourse.bass as bass
import concourse.mybir as mybir
import concourse.tile as tile
from concourse.bass_utils import run_bass_kernel_spmd

F32 = mybir.dt.float32
BF16 = mybir.dt.bfloat16
BF_NP = ml_dtypes.bfloat16

N, M, G, A, H = 20000, 48, 16, 64, 16
NCORES = 8
NL = N // NCORES  # 2500 atoms per core
NPAIRS = NL // 2  # 1250; atom n = 2p + s  (s = col-tile set)
GP = 256  # pairs per group (512 atoms): stage-2/epilogue granularity
BLK = 64  # pairs per input DMA block (128 atoms)
FL = 8  # pairs per psum1 flush (16 atoms, 1 psum bank)

# vbig partition row map: set s at rows 64s..64s+64:
#   rows 64s+0:16   out_s^T (g)
#   rows 64s+16+16d : 16  t d-slice (g)
# Stage-2 matmuls use K=32 (a full 32-row group) with zero-padded weights so
# lhsT/rhs partition bases stay 32-aligned (walrus requires base == tile row
# base). Row-group base per (s, d); weight variant (column offset) per d:
# A (offset 0) holds agh at the d0/d1 row offsets, B (offset 1536) at d2's.
RB32 = {(0, 0): 0, (0, 1): 32, (0, 2): 32, (1, 0): 64, (1, 1): 96, (1, 2): 96}
WOFF = {0: 0, 1: 0, 2: 1536}  # d -> column offset into aghw
AGHW_COLS = 3072  # two variants x 32 pairs x 48 cols ([agh_a |0| agh_b])

_nc_cache = {}


def _dummy_mm(nc, out_ap, dep_ap):
    # 1x1 matmul whose only job is to absorb one semaphore wait (walrus
    # encodes <=1 sync-wait per PE instruction); the dependency is expressed
    # through its operands (dep_ap must sit at partition base 0).
    nc.tensor.matmul(
        out=out_ap, lhsT=dep_ap, rhs=dep_ap, start=True, stop=True,
        tile_position=(0, 0),
    )


def _build(nl=NL, sim=False):
    """Per-core Bass program (bf16 end to end, fp32 PSUM accumulate).

    Stage 1 (per atom): one matmul, lhsT = acat cols 64:128 ([gs16|gv48]),
      rhs = acat cols 0:64 (a). out[0:16]=out_s^T, out[16:64]=t, with even
      atoms on out partitions 0:64 and odd on 64:128 (PE col tiling), so the
      PSUM->SBUF copy runs on all 128 partitions.
    Stage 2 (per 256-pair group, per channel/set/d): K=16 matmul of agh
      against vbig t-rows, 12-way tile_position packing, f = pairs.
    Epilogue: ACT Square psum->sbuf bf16, 2 DVE adds (d-sum), DMA out.
    """
    nc = bacc.Bacc("TRN2", target_bir_lowering=False)
    npairs = nl // 2
    ac_d = nc.declare_dram_parameter("ac", [M, nl, 128], BF16, isOutput=False)
    w_d = nc.declare_dram_parameter("aghw", [128, AGHW_COLS], BF16, isOutput=False)
    outs_d = nc.declare_dram_parameter(
        "outs", [2, G, npairs, A], BF16, isOutput=True
    )
    outv_d = nc.declare_dram_parameter(
        "outv", [2, A, H, npairs], BF16, isOutput=True
    )
    Sq = mybir.ActivationFunctionType.Square

    groups = []
    p = 0
    while p < npairs:
        groups.append((p, min(GP, npairs - p)))
        p += GP

    with tile.TileContext(nc) as tc:
        with (
            tc.tile_pool(name="singles", bufs=1) as singles,
            tc.tile_pool(name="acs", bufs=2) as acs_pool,
            tc.tile_pool(name="vbig", bufs=2) as vbig_pool,
            tc.tile_pool(name="sq", bufs=2) as sq_pool,
            tc.tile_pool(name="ov", bufs=2) as ov_pool,
            tc.tile_pool(name="psum1", bufs=2, space="PSUM") as p1_pool,
            tc.tile_pool(name="psum2", bufs=2, space="PSUM") as p2_pool,
        ):
            aghw = singles.tile([128, AGHW_COLS], BF16)
            nc.sync.dma_start(out=aghw[:, :], in_=w_d[:, :])

            def stage1(p0, npg, flush_ctr):
                vbig = vbig_pool.tile([128, GP * A], BF16)
                last_act_col = [None]
                for b0 in range(0, npg, BLK):
                    nb = min(BLK, npg - b0)
                    acs = acs_pool.tile([128, BLK * 2 * 128], BF16)
                    nc.sync.dma_start(
                        out=acs[0:M, 0 : nb * 256],
                        in_=ac_d[:, (p0 + b0) * 2 : (p0 + b0 + nb) * 2, :].rearrange(
                            "m n c -> m (n c)"
                        ),
                    )
                    for f0 in range(0, nb, FL):
                        nf = min(FL, nb - f0)
                        psum1 = p1_pool.tile([128, FL * A], F32)
                        if f0 == 0:
                            # absorb the psum1 WAR wait into a dummy (its
                            # aghw dep is wait-dominated after the first
                            # use), so the first real matmul's single wait
                            # is the fresh acs-DMA semaphore
                            _dummy_mm(nc, psum1[0:1, 0:1], aghw[0:1, 0:1])
                        for j in range(nf):
                            col = (f0 + j) * 256
                            nc.tensor.matmul(
                                out=psum1[0:64, j * A : (j + 1) * A],
                                lhsT=acs[0:M, col + 64 : col + 128],
                                rhs=acs[0:M, col : col + 64],
                                start=True,
                                stop=True,
                            )
                            nc.tensor.matmul(
                                out=psum1[64:128, j * A : (j + 1) * A],
                                lhsT=acs[0:M, col + 192 : col + 256],
                                rhs=acs[0:M, col + 128 : col + 192],
                                start=True,
                                stop=True,
                            )
                        # spread the PSUM->SBUF copies over DVE and ACT
                        # (3:2) to balance engine load
                        on_act = flush_ctr[0] % 5 >= 3
                        flush_ctr[0] += 1
                        dst = vbig[:, (b0 + f0) * A : (b0 + f0 + nf) * A]
                        if on_act:
                            nc.scalar.copy(out=dst, in_=psum1[:, 0 : nf * A])
                            last_act_col[0] = (b0 + f0) * A
                        else:
                            nc.vector.tensor_copy(
                                out=dst, in_=psum1[:, 0 : nf * A]
                            )
                # out_s: vbig rows {0:16, 64:80} -> [2, G, npairs, A]
                # (dst pre-merged to one 32KB run per partition)
                for s2 in range(2):
                    nc.sync.dma_start(
                        out=outs_d[s2].rearrange("g p a -> g (p a)")[
                            :, p0 * A : (p0 + npg) * A
                        ],
                        in_=vbig[64 * s2 : 64 * s2 + 16, 0 : npg * A],
                    )
                return vbig, last_act_col[0]

            def stage2(p0, npg, vbig, act_col, first_group):
                ov = ov_pool.tile([128, 16 * GP], BF16)
                for q in range(8):
                    for s in range(2):
                        # d-slices at 512-col offsets: one PSUM bank per d,
                        # so concurrent row-group tiles drain to distinct
                        # banks (same-bank cross-row-group drains are fatal)
                        psum2 = p2_pool.tile([128, 3 * 512], F32)
                        if q == 0 and s == 0:
                            # absorb the psum2 WAR wait (ACT square of 2
                            # iters ago) into a dummy targeting this psum2
                            # tile; that ACT wait also dominates the ACT
                            # copies into vbig (emitted earlier on the ACT
                            # queue), so the first real matmul's single wait
                            # is the DVE-copies-done semaphore. For the very
                            # first group there is no prior square, so the
                            # dummy instead reads the last ACT-copied vbig
                            # cell to carry the ACT dependency.
                            dep = (
                                vbig[0:1, act_col : act_col + 1]
                                if first_group and act_col is not None
                                else aghw[0:1, 0:1]
                            )
                            _dummy_mm(nc, psum2[0:1, 0:1], dep)
                        for d in range(3):
                            rb = RB32[(s, d)]
                            woff = WOFF[d]
                            for k in range(4):  # channel pairs in this q
                                c = 32 * k
                                pair = 4 * q + k
                                a0 = 8 * q + 2 * k  # alpha channel
                                out_ap = psum2[c : c + 32, d * 512 : d * 512 + npg]
                                rhs3 = vbig[rb : rb + 32, 0 : npg * A].rearrange(
                                    "r (p a) -> r p a", a=A
                                )
                                nc.tensor.matmul(
                                    out=out_ap,
                                    lhsT=aghw[
                                        rb : rb + 32,
                                        woff + 48 * pair : woff + 48 * pair + 32,
                                    ],
                                    rhs=rhs3[:, :, a0],
                                    start=True,
                                    stop=False,
                                    tile_position=(rb, c),
                                )
                                nc.tensor.matmul(
                                    out=out_ap,
                                    lhsT=aghw[
                                        rb : rb + 32,
                                        woff + 48 * pair + 16 : woff + 48 * pair + 48,
                                    ],
                                    rhs=rhs3[:, :, a0 + 1],
                                    start=False,
                                    stop=True,
                                    tile_position=(rb, c),
                                )
                        sq = sq_pool.tile([128, 3 * GP], BF16)
                        nc.scalar.activation(
                            out=sq[:, 0 : 3 * npg],
                            in_=psum2[:, :].rearrange(
                                "p (d z) -> p d z", z=512
                            )[:, :, 0:npg],
                            func=Sq,
                        )
                        ovs = ov[:, (q * 2 + s) * npg : (q * 2 + s + 1) * npg]
                        nc.vector.tensor_add(
                            ovs, sq[:, 0:npg], sq[:, npg : 2 * npg]
                        )
                        nc.vector.tensor_add(
                            ovs, ovs, sq[:, 2 * npg : 3 * npg]
                        )
                for s in range(2):
                    dst = outv_d[s, :, :, p0 : p0 + npg].rearrange(
                        "(q j) h n -> (j h) q n", j=8
                    )
                    src = ov[:, 0 : 16 * npg].rearrange(
                        "p (q s n) -> p q s n", q=8, s=2
                    )[:, :, s, :]
                    nc.sync.dma_start(out=dst, in_=src)

            with nc.allow_low_precision("bf16 stores; fp32 psum accumulate"):
                flush_ctr = [0]
                prev = None
                for gi, (p0, npg) in enumerate(groups):
                    vb, acol = stage1(p0, npg, flush_ctr)
                    if prev is not None:
                        stage2(*prev, first_group=(gi == 1))
                    prev = (p0, npg, vb, acol)
                stage2(*prev, first_group=(len(groups) == 1))
    nc.compile()
    return nc


def _get_nc():
    if "nc" not in _nc_cache:
        _nc_cache["nc"] = _build()
    return _nc_cache["nc"]


def _prep(a, gs, gv, agh):
    a = np.asarray(a, np.float32)
    gs = np.asarray(gs, np.float32)
    gv = np.asarray(gv, np.float32)
    agh = np.asarray(agh, np.float32)
    n = a.shape[0]
    ac = np.empty((M, n, 128), BF_NP)
    ac[:, :, 0:64] = a.transpose(1, 0, 2)
    ac[:, :, 64:80] = gs.transpose(1, 0, 2)
    # c = 80 + 16d + g  (d-major)
    ac[:, :, 80:128] = gv.transpose(1, 0, 3, 2).reshape(M, n, 48)
    # aghw [128, 3072]: variant A cols [0:1536] (d0/d1 row offsets), variant
    # B cols [1536:3072] (d2). Per channel pair: [agh_a(16) | 0(16) | agh_b].
    blkcols = np.zeros((G, 1536), np.float32)
    for pair in range(32):
        blkcols[:, 48 * pair : 48 * pair + 16] = agh[2 * pair]
        blkcols[:, 48 * pair + 32 : 48 * pair + 48] = agh[2 * pair + 1]
    w = np.zeros((128, AGHW_COLS), BF_NP)
    for r in (16, 32, 80, 96):  # variant A: d0 (offset 16), d1 (offset 0)
        w[r : r + 16, 0:1536] = blkcols
    for r in (48, 112):  # variant B: d2 (offset 16 in grp1/grp3)
        w[r : r + 16, 1536:3072] = blkcols
    return ac, w


def _assemble(res_core, nl):
    npairs = nl // 2
    o_s = (
        np.asarray(res_core["outs"])
        .astype(np.float32)
        .transpose(2, 0, 3, 1)
        .reshape(nl, A * G)
    )
    o_v = (
        np.asarray(res_core["outv"])
        .astype(np.float32)
        .transpose(3, 0, 1, 2)
        .reshape(nl, A * H)
    )
    return np.concatenate([o_s, o_v], axis=1)


def kernel(a, gs, gv, agh):
    ac, w = _prep(a, gs, gv, agh)
    nc = _get_nc()
    in_maps = [
        {"ac": np.ascontiguousarray(ac[:, c * NL : (c + 1) * NL, :]), "aghw": w}
        for c in range(NCORES)
    ]
    res = run_bass_kernel_spmd(nc, in_maps, list(range(NCORES))).results
    return np.concatenate(
        [_assemble(res[c], NL) for c in range(NCORES)], axis=0
    )


# revision 32
# speedup vs baseline: 1.0217x; 1.0202x over previous
import sys

sys.path.insert(0, "/opt/trn_rl_repo")

import ml_dtypes
import numpy as np

import concourse.bacc as bacc
import concourse.bass as bass
import concourse.mybir as mybir
import concourse.tile as tile
from concourse.bass_utils import run_bass_kernel_spmd

F32 = mybir.dt.float32
BF16 = mybir.dt.bfloat16
BF_NP = ml_dtypes.bfloat16

N, M, G, A, H = 20000, 48, 16, 64, 16
NCORES = 8
NL = N // NCORES  # 2500 atoms per core
NPAIRS = NL // 2  # 1250; atom n = 2p + s  (s = col-tile set)
GP = 256  # pairs per group (512 atoms): stage-2/epilogue granularity
BLK = 32  # pairs per input DMA block (64 atoms)
FL = 8  # pairs per psum1 flush (16 atoms, 1 psum bank)

# vbig partition row map: set s at rows 64s..64s+64:
#   rows 64s+0:16   out_s^T (g)
#   rows 64s+16+16d : 16  t d-slice (g)
# Stage-2 matmuls use K=32 (a full 32-row group) with zero-padded weights so
# lhsT/rhs partition bases stay 32-aligned (walrus requires base == tile row
# base). Row-group base per (s, d); weight variant (column offset) per d:
# A (offset 0) holds agh at the d0/d1 row offsets, B (offset 1536) at d2's.
RB32 = {(0, 0): 0, (0, 1): 32, (0, 2): 32, (1, 0): 64, (1, 1): 96, (1, 2): 96}
WOFF = {0: 0, 1: 0, 2: 1536}  # d -> column offset into aghw
AGHW_COLS = 3072  # two variants x 32 pairs x 48 cols ([agh_a |0| agh_b])

_nc_cache = {}


def _dummy_mm(nc, out_ap, dep_ap):
    # 1x1 matmul whose only job is to absorb one semaphore wait (walrus
    # encodes <=1 sync-wait per PE instruction); the dependency is expressed
    # through its operands (dep_ap must sit at partition base 0).
    nc.tensor.matmul(
        out=out_ap, lhsT=dep_ap, rhs=dep_ap, start=True, stop=True,
        tile_position=(0, 0),
    )


def _build(nl=NL, sim=False):
    """Per-core Bass program (bf16 end to end, fp32 PSUM accumulate).

    Stage 1 (per atom): one matmul, lhsT = acat cols 64:128 ([gs16|gv48]),
      rhs = acat cols 0:64 (a). out[0:16]=out_s^T, out[16:64]=t, with even
      atoms on out partitions 0:64 and odd on 64:128 (PE col tiling), so the
      PSUM->SBUF copy runs on all 128 partitions.
    Stage 2 (per 256-pair group, per channel/set/d): K=16 matmul of agh
      against vbig t-rows, 12-way tile_position packing, f = pairs.
    Epilogue: ACT Square psum->sbuf bf16, 2 DVE adds (d-sum), DMA out.
    """
    nc = bacc.Bacc("TRN2", target_bir_lowering=False)
    npairs = nl // 2
    ac_d = nc.declare_dram_parameter("ac", [M, nl, 128], BF16, isOutput=False)
    w_d = nc.declare_dram_parameter("aghw", [128, AGHW_COLS], BF16, isOutput=False)
    outs_d = nc.declare_dram_parameter(
        "outs", [2, G, npairs, A], BF16, isOutput=True
    )
    outv_d = nc.declare_dram_parameter(
        "outv", [2, A, H, npairs], BF16, isOutput=True
    )
    Sq = mybir.ActivationFunctionType.Square

    groups = []
    p = 0
    while p < npairs:
        groups.append((p, min(GP, npairs - p)))
        p += GP

    with tile.TileContext(nc) as tc:
        with (
            tc.tile_pool(name="singles", bufs=1) as singles,
            tc.tile_pool(name="acs", bufs=6) as acs_pool,
            tc.tile_pool(name="vbig", bufs=2) as vbig_pool,
            tc.tile_pool(name="sq", bufs=2) as sq_pool,
            tc.tile_pool(name="ov", bufs=2) as ov_pool,
            tc.tile_pool(name="psum1", bufs=2, space="PSUM") as p1_pool,
            tc.tile_pool(name="psum2", bufs=2, space="PSUM") as p2_pool,
        ):
            aghw = singles.tile([128, AGHW_COLS], BF16)
            nc.sync.dma_start(out=aghw[:, :], in_=w_d[:, :])

            def stage1(p0, npg, flush_ctr):
                vbig = vbig_pool.tile([128, GP * A], BF16)
                last_act_col = [None]
                for b0 in range(0, npg, BLK):
                    nb = min(BLK, npg - b0)
                    acs = acs_pool.tile([128, BLK * 2 * 128], BF16)
                    nc.sync.dma_start(
                        out=acs[0:M, 0 : nb * 256],
                        in_=ac_d[:, (p0 + b0) * 2 : (p0 + b0 + nb) * 2, :].rearrange(
                            "m n c -> m (n c)"
                        ),
                    )
                    for f0 in range(0, nb, FL):
                        nf = min(FL, nb - f0)
                        psum1 = p1_pool.tile([128, FL * A], F32)
                        if f0 == 0:
                            # absorb the psum1 WAR wait into a dummy (its
                            # aghw dep is wait-dominated after the first
                            # use), so the first real matmul's single wait
                            # is the fresh acs-DMA semaphore
                            _dummy_mm(nc, psum1[0:1, 0:1], aghw[0:1, 0:1])
                        for j in range(nf):
                            col = (f0 + j) * 256
                            nc.tensor.matmul(
                                out=psum1[0:64, j * A : (j + 1) * A],
                                lhsT=acs[0:M, col + 64 : col + 128],
                                rhs=acs[0:M, col : col + 64],
                                start=True,
                                stop=True,
                            )
                            nc.tensor.matmul(
                                out=psum1[64:128, j * A : (j + 1) * A],
                                lhsT=acs[0:M, col + 192 : col + 256],
                                rhs=acs[0:M, col + 128 : col + 192],
                                start=True,
                                stop=True,
                            )
                        # spread the PSUM->SBUF copies over DVE and ACT
                        # (3:2) to balance engine load
                        on_act = flush_ctr[0] % 5 >= 3
                        flush_ctr[0] += 1
                        dst = vbig[:, (b0 + f0) * A : (b0 + f0 + nf) * A]
                        if on_act:
                            nc.scalar.copy(out=dst, in_=psum1[:, 0 : nf * A])
                            last_act_col[0] = (b0 + f0) * A
                        else:
                            nc.vector.tensor_copy(
                                out=dst, in_=psum1[:, 0 : nf * A]
                            )
                return vbig, last_act_col[0]

            def outs_dma(p0, npg, vbig):
                # out_s: vbig rows {0:16, 64:80} -> [2, G, npairs, A]
                # (dst pre-merged to one 32KB run per partition). Emitted a
                # group late so its waits are satisfied when SP reaches it
                # and input prefetch is never blocked behind it.
                for s2 in range(2):
                    nc.sync.dma_start(
                        out=outs_d[s2].rearrange("g p a -> g (p a)")[
                            :, p0 * A : (p0 + npg) * A
                        ],
                        in_=vbig[64 * s2 : 64 * s2 + 16, 0 : npg * A],
                    )

            def stage2(p0, npg, vbig, act_col, first_group):
                ov = ov_pool.tile([128, 16 * GP], BF16)
                for q in range(8):
                    for s in range(2):
                        # d-slices at 512-col offsets: one PSUM bank per d,
                        # so concurrent row-group tiles drain to distinct
                        # banks (same-bank cross-row-group drains are fatal)
                        psum2 = p2_pool.tile([128, 3 * 512], F32)
                        if q == 0 and s == 0:
                            # absorb the psum2 WAR wait (ACT square of 2
                            # iters ago) into a dummy targeting this psum2
                            # tile; that ACT wait also dominates the ACT
                            # copies into vbig (emitted earlier on the ACT
                            # queue), so the first real matmul's single wait
                            # is the DVE-copies-done semaphore. For the very
                            # first group there is no prior square, so the
                            # dummy instead reads the last ACT-copied vbig
                            # cell to carry the ACT dependency.
                            dep = (
                                vbig[0:1, act_col : act_col + 1]
                                if first_group and act_col is not None
                                else aghw[0:1, 0:1]
                            )
                            _dummy_mm(nc, psum2[0:1, 0:1], dep)
                        for d in range(3):
                            rb = RB32[(s, d)]
                            woff = WOFF[d]
                            for k in range(4):  # channel pairs in this q
                                c = 32 * k
                                pair = 4 * q + k
                                a0 = 8 * q + 2 * k  # alpha channel
                                out_ap = psum2[c : c + 32, d * 512 : d * 512 + npg]
                                rhs3 = vbig[rb : rb + 32, 0 : npg * A].rearrange(
                                    "r (p a) -> r p a", a=A
                                )
                                nc.tensor.matmul(
                                    out=out_ap,
                                    lhsT=aghw[
                                        rb : rb + 32,
                                        woff + 48 * pair : woff + 48 * pair + 32,
                                    ],
                                    rhs=rhs3[:, :, a0],
                                    start=True,
                                    stop=False,
                                    tile_position=(rb, c),
                                )
                                nc.tensor.matmul(
                                    out=out_ap,
                                    lhsT=aghw[
                                        rb : rb + 32,
                                        woff + 48 * pair + 16 : woff + 48 * pair + 48,
                                    ],
                                    rhs=rhs3[:, :, a0 + 1],
                                    start=False,
                                    stop=True,
                                    tile_position=(rb, c),
                                )
                        sq = sq_pool.tile([128, 3 * GP], BF16)
                        nc.scalar.activation(
                            out=sq[:, 0 : 3 * npg],
                            in_=psum2[:, :].rearrange(
                                "p (d z) -> p d z", z=512
                            )[:, :, 0:npg],
                            func=Sq,
                        )
                        ovs = ov[:, (q * 2 + s) * npg : (q * 2 + s + 1) * npg]
                        nc.vector.tensor_add(
                            ovs, sq[:, 0:npg], sq[:, npg : 2 * npg]
                        )
                        nc.vector.tensor_add(
                            ovs, ovs, sq[:, 2 * npg : 3 * npg]
                        )
                return ov

            def outv_dma(p0, npg, ov):
                # emitted two groups late (adds long done by then)
                for s in range(2):
                    dst = outv_d[s, :, :, p0 : p0 + npg].rearrange(
                        "(q j) h n -> (j h) q n", j=8
                    )
                    src = ov[:, 0 : 16 * npg].rearrange(
                        "p (q s n) -> p q s n", q=8, s=2
                    )[:, :, s, :]
                    nc.sync.dma_start(out=dst, in_=src)

            with nc.allow_low_precision("bf16 stores; fp32 psum accumulate"):
                flush_ctr = [0]
                done = []  # per group: (p0, npg, vbig, acol, ov-or-None)
                for gi, (p0, npg) in enumerate(groups):
                    vb, acol = stage1(p0, npg, flush_ctr)
                    if gi >= 1:
                        outs_dma(*done[gi - 1][:3])
                    if gi >= 2:
                        g2 = done[gi - 2]
                        outv_dma(g2[0], g2[1], g2[4])
                    if gi >= 1:
                        gp = done[gi - 1]
                        ov = stage2(
                            gp[0], gp[1], gp[2], gp[3],
                            first_group=(gi == 1),
                        )
                        done[gi - 1] = gp[:4] + (ov,)
                    done.append((p0, npg, vb, acol, None))
                ng = len(groups)
                gl = done[ng - 1]
                outs_dma(*gl[:3])
                if ng >= 2:
                    g2 = done[ng - 2]
                    outv_dma(g2[0], g2[1], g2[4])
                ov = stage2(gl[0], gl[1], gl[2], gl[3], first_group=(ng == 1))
                outv_dma(gl[0], gl[1], ov)
    nc.compile()
    return nc


def _get_nc():
    if "nc" not in _nc_cache:
        _nc_cache["nc"] = _build()
    return _nc_cache["nc"]


def _prep(a, gs, gv, agh):
    a = np.asarray(a, np.float32)
    gs = np.asarray(gs, np.float32)
    gv = np.asarray(gv, np.float32)
    agh = np.asarray(agh, np.float32)
    n = a.shape[0]
    ac = np.empty((M, n, 128), BF_NP)
    ac[:, :, 0:64] = a.transpose(1, 0, 2)
    ac[:, :, 64:80] = gs.transpose(1, 0, 2)
    # c = 80 + 16d + g  (d-major)
    ac[:, :, 80:128] = gv.transpose(1, 0, 3, 2).reshape(M, n, 48)
    # aghw [128, 3072]: variant A cols [0:1536] (d0/d1 row offsets), variant
    # B cols [1536:3072] (d2). Per channel pair: [agh_a(16) | 0(16) | agh_b].
    blkcols = np.zeros((G, 1536), np.float32)
    for pair in range(32):
        blkcols[:, 48 * pair : 48 * pair + 16] = agh[2 * pair]
        blkcols[:, 48 * pair + 32 : 48 * pair + 48] = agh[2 * pair + 1]
    w = np.zeros((128, AGHW_COLS), BF_NP)
    for r in (16, 32, 80, 96):  # variant A: d0 (offset 16), d1 (offset 0)
        w[r : r + 16, 0:1536] = blkcols
    for r in (48, 112):  # variant B: d2 (offset 16 in grp1/grp3)
        w[r : r + 16, 1536:3072] = blkcols
    return ac, w


def _assemble(res_core, nl):
    npairs = nl // 2
    o_s = (
        np.asarray(res_core["outs"])
        .astype(np.float32)
        .transpose(2, 0, 3, 1)
        .reshape(nl, A * G)
    )
    o_v = (
        np.asarray(res_core["outv"])
        .astype(np.float32)
        .transpose(3, 0, 1, 2)
        .reshape(nl, A * H)
    )
    return np.concatenate([o_s, o_v], axis=1)


def kernel(a, gs, gv, agh):
    ac, w = _prep(a, gs, gv, agh)
    nc = _get_nc()
    in_maps = [
        {"ac": np.ascontiguousarray(ac[:, c * NL : (c + 1) * NL, :]), "aghw": w}
        for c in range(NCORES)
    ]
    res = run_bass_kernel_spmd(nc, in_maps, list(range(NCORES))).results
    return np.concatenate(
        [_assemble(res[c], NL) for c in range(NCORES)], axis=0
    )
